# revision 17
# baseline (speedup 1.0000x reference)
"""AdaptiveModulatedConv3d — 8-core TRN2 Bass kernel.

Problem (hardcoded): BS=8, C_IN=C_OUT=64, K=3, STYLE_DIM=512, BANK=4,
D=H=W=32, pad=1, stride=1, f32 in/out.

Sharding: pure data-parallel over batch — each of the 8 NeuronCores gets
one sample, builds its per-sample demodulated conv weights on-device, and
runs its own 3D conv. No collectives.

Per-core conv strategy: the 3x3x3 conv is decomposed into shifted matmuls
(contraction over C_IN=64) accumulating into PSUM. The PE 128x128 array is
quadrant-packed: row-groups 0/64 hold two copies of x (bf16, upper shifted
by +1 element), col-groups 0/64 compute two output tiles in separate PSUM
banks. Boundary taps (d and h) use narrowed-N matmuls instead of padding.

Latency layout: x ships from the host already padded + bf16 (both shifted
copies in one HBM buffer, chunk DMAs straight into SBUF). The per-sample
mix scalars fm[ci,n] = softmax_n(w@filter_w.T+fb)*mod[ci] are tiny
(BANK*CI values) and ride along as marshalled input; the bank arrives in
per-bank sub-DMAs so the DVE mix (bf16, two koff pieces) starts the moment
the first sub-bank lands. d-planes are visited in order [1,2,0,3,4,...] so
group 0 only needs the first WT piece. The demodulation (sum-of-squares
matmuls + rsqrt) is emitted interleaved into the early wave stream. Drains
read PSUM strided into compact SBUF tiles (fully contiguous store DMAs),
alternating ACT/DVE engines and GpSimd/SP issue queues.
"""

import numpy as np

import concourse.bass as bass
import concourse.tile as tile
from concourse import bacc, mybir
from concourse import bass_utils

F32 = mybir.dt.float32
BF16 = mybir.dt.bfloat16

BS = 8
CI = 64
CO = 64
SD = 512
BANK = 4
D = H = W = 32
EPS = 1e-8
NCORES = 8

PLANE = (H + 2) * (W + 2)  # 1156: h/w padded plane, flattened
XCOLS = 3 + D * PLANE
ROWSPLIT = [(0, 11), (11, 11), (22, 10)]  # h-row tiles per d-plane
KSPLIT = 18  # mix piece boundary (koff)
XCHUNKS = [(0, 2), (2, 4), (4, 8), (8, 12), (12, 16), (16, 20), (20, 24),
           (24, 28), (28, 32)]
D_ORDER = [1, 2, 0] + list(range(3, D))

_CACHE = {}


def _tile_taps(d, r0, nr):
    """Valid taps for tile (d, r0, nr) with h-boundary narrowing: rows
    whose x_pad source row is padding are excluded (their contribution is
    zero), so no row-border zeroing is ever needed."""
    taps = []
    for kd in range(3):
        if not (0 <= d + kd - 1 <= D - 1):
            continue
        for kh in range(3):
            rlo = max(r0, 1 - kh)
            rhi = min(r0 + nr - 1, 32 - kh)
            for kw in range(3):
                taps.append((kd, kh, kw, rlo, rhi - rlo + 1))
    return taps


def _build():
    nc = bacc.Bacc("TRN2", target_bir_lowering=False, debug=False)
    xpad = nc.dram_tensor("xpad", [128, XCOLS], BF16,
                          kind="ExternalInput").ap()
    fmh = nc.dram_tensor("fmh", [128, BANK], F32, kind="ExternalInput").ap()
    bankt = nc.dram_tensor("bankt", [128, BANK, 27 * CO], BF16,
                           kind="ExternalInput").ap()
    out = nc.dram_tensor("out", [CO, D, H, W], F32, kind="ExternalOutput").ap()

    AF = mybir.ActivationFunctionType
    MULT, ADD = mybir.AluOpType.mult, mybir.AluOpType.add
    KS = KSPLIT

    with tile.TileContext(nc) as tc:
        with tc.tile_pool(name="singles", bufs=1) as singles, \
             tc.tile_pool(name="osb", bufs=10) as osb_pool:

            # fm and bank arrive duplicated on both partition halves, so
            # the mix runs on all 128 DVE lanes and writes both WT copies
            # at once (no upper-half dup DMA). DMA issue order on the sync
            # queue puts the mix's critical chain ahead of the bulk of x.
            fm_sb = singles.tile([128, BANK], F32)
            nc.sync.dma_start(out=fm_sb, in_=fmh)
            bank_sb = singles.tile([128, BANK, 27 * CO], BF16)
            xbf = singles.tile([128, XCOLS], BF16)

            def emit_xchunk(ci):
                p0, p1 = XCHUNKS[ci]
                a = 0 if p0 == 0 else 1 + p0 * PLANE
                b = XCOLS if p1 == D else 2 + p1 * PLANE
                nc.sync.dma_start(out=xbf[:, a:b], in_=xpad[:, a:b])

            # bank piece-0 sub-DMAs split across the sync and ACT issue
            # queues (issue cost ~0.7us each serializes per queue); x
            # chunks 0-1 and bank piece 1 ride ACT, the rest sync
            def bank_n(eng, n):
                eng.dma_start(out=bank_sb[:, n, 0:KS * CO],
                              in_=bankt[:, n, 0:KS * CO])

            bank_n(nc.sync, 0)
            bank_n(nc.scalar, 2)
            bank_n(nc.sync, 1)
            bank_n(nc.scalar, 3)
            nc.scalar.dma_start(out=xbf[:, 0:2 + 2 * PLANE],
                                in_=xpad[:, 0:2 + 2 * PLANE])
            nc.scalar.dma_start(
                out=xbf[:, 1 + 2 * PLANE:2 + 4 * PLANE],
                in_=xpad[:, 1 + 2 * PLANE:2 + 4 * PLANE])
            nc.scalar.dma_start(out=bank_sb[:, :, KS * CO:27 * CO],
                                in_=bankt[:, :, KS * CO:27 * CO])
            for ci in range(2, len(XCHUNKS)):
                emit_xchunk(ci)

            warm = singles.tile([1, 1], F32)
            nc.vector.memset(warm, 0.0)
            ones1 = singles.tile([1, 64], F32)
            nc.vector.memset(ones1, 1.0)
            ones64 = singles.tile([64, 1], BF16)
            nc.vector.memset(ones64, 1.0)
            eps_sb = singles.tile([1, 1], F32)
            nc.vector.memset(eps_sb, EPS)
            wdum = singles.tile([1, 512], BF16)
            nc.vector.memset(wdum, 1.0)
            nc.scalar.activation(warm, warm, AF.Sqrt)  # table warm

            # PE warm-up: ~4us of dummy matmuls releases the HAM clock
            # throttle (1.2 -> 2.4 GHz) before the real conv stream starts
            with tc.tile_pool(name="hpsum", bufs=1, space="PSUM") as hpsum:
                hps = hpsum.tile([1, 512], F32, tag="hps")
                for _ in range(11):
                    nc.tensor.matmul(hps, lhsT=ones64[0:1, 0:1], rhs=wdum,
                                     start=True, stop=True)

            WT = singles.tile([128, 27, CO], BF16)
            sq = singles.tile([CI, 27 * CO], BF16)
            sstd = singles.tile([1, CO], F32)
            demod_sb = singles.tile([1, CO], F32)
            dmT = singles.tile([CO, 1], F32)

            # bank mix on all 128 DVE lanes: WT[ci(+64),koff,co] =
            # sum_n fm[ci,n]*bank[ci,n,...], bf16; piece sq right after
            for (k0, k1) in ((0, KS), (KS, 27)):
                f0, f1 = k0 * CO, k1 * CO
                WTf = WT[:, k0:k1].rearrange("p k c -> p (k c)")
                nc.vector.tensor_scalar_mul(WTf, bank_sb[:, 0, f0:f1],
                                            fm_sb[:, 0:1])
                for n in range(1, 4):
                    nc.vector.scalar_tensor_tensor(
                        out=WTf, in0=bank_sb[:, n, f0:f1],
                        scalar=fm_sb[:, n:n + 1], in1=WTf,
                        op0=MULT, op1=ADD)
                nc.vector.tensor_mul(sq[:, f0:f1], WTf[0:64], WTf[0:64])

            # ---- conv ----
            tiles_l = [(d, r0, nr) for d in D_ORDER for (r0, nr) in ROWSPLIT]
            quads = [(0, 0), (64, 0), (0, 64), (64, 64)]
            with tc.tile_pool(name="cpsum", bufs=8, space="PSUM") as cpsum:
                # demod scratch claims slot 0 so group slot reuse distance
                # stays >= 2 groups
                dps = cpsum.tile([128, 512], F32, tag="cps", name="dps")
                for ti in range(0, len(tiles_l), 4):
                    g = ti // 4
                    group = tiles_l[ti:ti + 4]
                    pss = [cpsum.tile([128, 512], F32, tag="cps",
                                      name=f"cps{j}")
                           for j in range(len(group))]
                    osbA = osb_pool.tile([128, 2, 11, W], F32, name="osbA")
                    osbs = [osbA[0:64, 0], osbA[0:64, 1],
                            osbA[64:128, 0], osbA[64:128, 1]]
                    taps_l = [_tile_taps(d, r0, nr) for (d, r0, nr) in group]
                    nwaves = max(len(t) for t in taps_l)
                    for i in range(nwaves):
                        if g == 0 and i == 14:
                            # demod sums over piece 0 (sq cols ready)
                            for k in range(KS):
                                nc.tensor.matmul(
                                    dps[0:1, 0:CO], lhsT=ones64,
                                    rhs=sq[:, k * CO:(k + 1) * CO],
                                    start=(k == 0), stop=False)
                        if g == 0 and i == 26:
                            for k in range(KS, 27):
                                nc.tensor.matmul(
                                    dps[0:1, 0:CO], lhsT=ones64,
                                    rhs=sq[:, k * CO:(k + 1) * CO],
                                    start=False, stop=(k == 26))
                            nc.scalar.activation(sstd, dps[0:1, 0:CO],
                                                 AF.Sqrt,
                                                 bias=eps_sb[:, 0:1])
                            nc.vector.reciprocal(demod_sb, sstd)
                        for j, (d, r0, nr) in enumerate(group):
                            taps = taps_l[j]
                            if i >= len(taps):
                                continue
                            kd, kh, kw, rlo, rcnt = taps[i]
                            rg, cp = quads[j]
                            koff = kd * 9 + kh * 3 + kw
                            n = rcnt * 34
                            c0 = (rlo - r0) * 34
                            off = (2 + (d + kd - 1) * PLANE
                                   + (rlo + kh) * 34 + kw - 1)
                            if rg:
                                off -= 1
                            nc.tensor.matmul(
                                pss[j][cp:cp + 64, c0:c0 + n],
                                lhsT=WT[rg:rg + 64, koff, :],
                                rhs=xbf[rg:rg + 64, off:off + n],
                                start=(i == 0), stop=(i == len(taps) - 1))
                    if g == 0:
                        # transpose demod to a per-partition column before
                        # this group's drains need it
                        nc.tensor.matmul(dps[0:CO, CO:CO + 1],
                                         lhsT=demod_sb, rhs=ones1[:, 0:1],
                                         start=True, stop=True)
                        nc.vector.tensor_copy(dmT, dps[0:CO, CO:CO + 1])
                    # drain: strided PSUM read -> compact SBUF -> one
                    # fully-contiguous store DMA per tile; engines and
                    # issue queues alternate to halve the serial cost
                    for j, (d, r0, nr) in enumerate(group):
                        cp = quads[j][1]
                        src = pss[j][cp:cp + 64, 0:nr * 34].rearrange(
                            "p (a b) -> p a b", b=34)[:, :, 1:W + 1]
                        if j >= 2:
                            nc.vector.tensor_scalar_mul(
                                osbs[j][:, 0:nr], src, dmT[:, 0:1])
                            nc.sync.dma_start(
                                out=out[:, d, r0:r0 + nr, :],
                                in_=osbs[j][:, 0:nr])
                        else:
                            nc.scalar.mul(osbs[j][:, 0:nr], src, dmT[:, 0:1])
                            nc.gpsimd.dma_start(
                                out=out[:, d, r0:r0 + nr, :],
                                in_=osbs[j][:, 0:nr])

    nc.compile()
    return nc


def _shard_inputs(x, w, filter_w, filter_b, mod_w, mod_b, bank):
    """Host-side input marshalling: per-core shards + replicated params in
    the layouts the kernel expects (padded bf16 x with both shifted
    copies; per-sample mix scalars; bf16 bank)."""
    import ml_dtypes
    wf = np.asarray(w, np.float32)
    logits = wf @ np.asarray(filter_w, np.float32).T + np.asarray(
        filter_b, np.float32)
    # softmax without the 1/sum: a uniform scale cancels through the
    # demodulation (exp of max-shifted logits for f32 safety)
    fwt = np.exp(logits - logits.max(axis=1, keepdims=True))  # [bs, bank]
    mod = wf @ np.asarray(mod_w, np.float32).T + np.asarray(
        mod_b, np.float32)  # [bs, ci]
    bank_h = np.ascontiguousarray(
        np.asarray(bank, np.float32).reshape(BANK, CO, CI, 27)
        .transpose(2, 0, 3, 1).reshape(CI, BANK, 27 * CO)
    ).astype(ml_dtypes.bfloat16)
    bank_h = np.ascontiguousarray(np.concatenate([bank_h, bank_h], axis=0))
    xf = np.asarray(x, np.float32)
    in_maps = []
    for i in range(NCORES):
        pad3 = np.zeros((CI, D, H + 2, W + 2), np.float32)
        pad3[:, :, 1:H + 1, 1:W + 1] = xf[i]
        fl = pad3.reshape(CI, -1).astype(ml_dtypes.bfloat16)
        xp = np.zeros((128, XCOLS), ml_dtypes.bfloat16)
        xp[0:64, 2:2 + D * PLANE] = fl
        xp[64:128, 1:1 + D * PLANE] = fl
        fm = mod[i][:, None] * fwt[i][None, :]  # [ci, bank]
        fm2 = np.ascontiguousarray(
            np.concatenate([fm, fm], axis=0), np.float32)
        in_maps.append({"xpad": xp, "fmh": fm2, "bankt": bank_h})
    return in_maps


def _run(inputs, trace=False):
    if "nc" not in _CACHE:
        _CACHE["nc"] = _build()
    nc = _CACHE["nc"]
    in_maps = _shard_inputs(**inputs)
    res = bass_utils.run_bass_kernel_spmd(
        nc, in_maps, core_ids=list(range(NCORES)), trace=trace)
    out = np.stack([res.results[i]["out"] for i in range(NCORES)])
    return out.astype(np.float32), res


def kernel(**inputs):
    out, _ = _run(inputs, trace=False)
    return out


# revision 20
# speedup vs baseline: 1.0095x; 1.0095x over previous
"""AdaptiveModulatedConv3d — 8-core TRN2 Bass kernel.

Problem (hardcoded): BS=8, C_IN=C_OUT=64, K=3, STYLE_DIM=512, BANK=4,
D=H=W=32, pad=1, stride=1, f32 in/out.

Sharding: pure data-parallel over batch — each of the 8 NeuronCores gets
one sample, builds its per-sample demodulated conv weights on-device, and
runs its own 3D conv. No collectives.

Per-core conv strategy: the 3x3x3 conv is decomposed into shifted matmuls
(contraction over C_IN=64) accumulating into PSUM. The PE 128x128 array is
quadrant-packed: row-groups 0/64 hold two copies of x (bf16, upper shifted
by +1 element), col-groups 0/64 compute two output tiles in separate PSUM
banks. Boundary taps (d and h) use narrowed-N matmuls instead of padding.

Latency layout: x ships from the host already padded + bf16 (both shifted
copies in one HBM buffer, chunk DMAs straight into SBUF). The per-sample
mix scalars fm[ci,n] = softmax_n(w@filter_w.T+fb)*mod[ci] are tiny
(BANK*CI values) and ride along as marshalled input; the bank arrives in
per-bank sub-DMAs so the DVE mix (bf16, two koff pieces) starts the moment
the first sub-bank lands. d-planes are visited in order [1,2,0,3,4,...] so
group 0 only needs the first WT piece. The demodulation (sum-of-squares
matmuls + rsqrt) is emitted interleaved into the early wave stream. Drains
read PSUM strided into compact SBUF tiles (fully contiguous store DMAs),
alternating ACT/DVE engines and GpSimd/SP issue queues.
"""

import numpy as np

import concourse.bass as bass
import concourse.tile as tile
from concourse import bacc, mybir
from concourse import bass_utils

F32 = mybir.dt.float32
BF16 = mybir.dt.bfloat16

BS = 8
CI = 64
CO = 64
SD = 512
BANK = 4
D = H = W = 32
EPS = 1e-8
NCORES = 8

PLANE = (H + 2) * (W + 2)  # 1156: h/w padded plane, flattened
XCOLS = 3 + D * PLANE
ROWSPLIT = [(0, 11), (11, 11), (22, 10)]  # h-row tiles per d-plane
KSPLIT = 18  # mix piece boundary (koff)
XCHUNKS = [(0, 2), (2, 4), (4, 8), (8, 12), (12, 16), (16, 20), (20, 24),
           (24, 28), (28, 32)]
D_ORDER = [1, 2, 0] + list(range(3, D))

_CACHE = {}


def _tile_taps(d, r0, nr):
    """Valid taps for tile (d, r0, nr) with h-boundary narrowing: rows
    whose x_pad source row is padding are excluded (their contribution is
    zero), so no row-border zeroing is ever needed."""
    taps = []
    for kd in range(3):
        if not (0 <= d + kd - 1 <= D - 1):
            continue
        for kh in range(3):
            rlo = max(r0, 1 - kh)
            rhi = min(r0 + nr - 1, 32 - kh)
            for kw in range(3):
                taps.append((kd, kh, kw, rlo, rhi - rlo + 1))
    return taps


def _build():
    nc = bacc.Bacc("TRN2", target_bir_lowering=False, debug=False)
    xpad = nc.dram_tensor("xpad", [128, XCOLS], BF16,
                          kind="ExternalInput").ap()
    fmh = nc.dram_tensor("fmh", [128, BANK], F32, kind="ExternalInput").ap()
    bankt = nc.dram_tensor("bankt", [128, BANK, 27 * CO], BF16,
                           kind="ExternalInput").ap()
    out = nc.dram_tensor("out", [CO, D, H, W], F32, kind="ExternalOutput").ap()

    AF = mybir.ActivationFunctionType
    MULT, ADD = mybir.AluOpType.mult, mybir.AluOpType.add
    KS = KSPLIT

    with tile.TileContext(nc) as tc:
        with tc.tile_pool(name="singles", bufs=1) as singles, \
             tc.tile_pool(name="osb", bufs=10) as osb_pool:

            # fm and bank arrive duplicated on both partition halves, so
            # the mix runs on all 128 DVE lanes and writes both WT copies
            # at once (no upper-half dup DMA). DMA issue order on the sync
            # queue puts the mix's critical chain ahead of the bulk of x.
            fm_sb = singles.tile([128, BANK], F32)
            nc.sync.dma_start(out=fm_sb, in_=fmh)
            bank_sb = singles.tile([128, BANK, 27 * CO], BF16)
            xbf = singles.tile([128, XCOLS], BF16)

            def emit_xchunk(ci):
                p0, p1 = XCHUNKS[ci]
                a = 0 if p0 == 0 else 1 + p0 * PLANE
                b = XCOLS if p1 == D else 2 + p1 * PLANE
                nc.sync.dma_start(out=xbf[:, a:b], in_=xpad[:, a:b])

            # bank piece-0 sub-DMAs split across the sync and ACT issue
            # queues (issue cost ~0.7us each serializes per queue); x
            # chunks 0-1 and bank piece 1 ride ACT, the rest sync
            def bank_n(eng, n):
                eng.dma_start(out=bank_sb[:, n, 0:KS * CO],
                              in_=bankt[:, n, 0:KS * CO])

            bank_n(nc.sync, 0)
            bank_n(nc.scalar, 2)
            bank_n(nc.sync, 1)
            bank_n(nc.scalar, 3)
            nc.scalar.dma_start(out=xbf[:, 0:2 + 2 * PLANE],
                                in_=xpad[:, 0:2 + 2 * PLANE])
            nc.scalar.dma_start(
                out=xbf[:, 1 + 2 * PLANE:2 + 4 * PLANE],
                in_=xpad[:, 1 + 2 * PLANE:2 + 4 * PLANE])
            nc.scalar.dma_start(out=bank_sb[:, :, KS * CO:27 * CO],
                                in_=bankt[:, :, KS * CO:27 * CO])
            for ci in range(2, len(XCHUNKS)):
                emit_xchunk(ci)
            tmix = singles.tile([128, 4, KS * CO], BF16)

            warm = singles.tile([1, 1], F32)
            nc.vector.memset(warm, 0.0)
            ones1 = singles.tile([1, 64], F32)
            nc.vector.memset(ones1, 1.0)
            ones64 = singles.tile([64, 1], BF16)
            nc.vector.memset(ones64, 1.0)
            eps_sb = singles.tile([1, 1], F32)
            nc.vector.memset(eps_sb, EPS)
            wdum = singles.tile([1, 512], BF16)
            nc.vector.memset(wdum, 1.0)
            nc.scalar.activation(warm, warm, AF.Sqrt)  # table warm

            # PE warm-up: ~3.5us of dummy matmuls releases the HAM clock
            # throttle (1.2 -> 2.4 GHz) just before the conv stream starts
            with tc.tile_pool(name="hpsum", bufs=1, space="PSUM") as hpsum:
                hps = hpsum.tile([1, 512], F32, tag="hps")
                for _ in range(8):
                    nc.tensor.matmul(hps, lhsT=ones64[0:1, 0:1], rhs=wdum,
                                     start=True, stop=True)

            WT = singles.tile([128, 27, CO], BF16)
            sq = singles.tile([CI, 27 * CO], BF16)
            sstd = singles.tile([1, CO], F32)
            demod_sb = singles.tile([1, CO], F32)
            dmT = singles.tile([CO, 1], F32)

            # bank mix on all 128 DVE lanes: WT[ci(+64),koff,co] =
            # sum_n fm[ci,n]*bank[ci,n,...], bf16. Mul-tree formulation:
            # tensor_scalar muls run at the DVE 4x / ACT packed rate and
            # bf16 tensor_tensor adds at 2x, vs 1x for the naive
            # scalar_tensor_tensor chain; the two ACT muls overlap DVE.
            for (k0, k1) in ((0, KS), (KS, 27)):
                f0, f1 = k0 * CO, k1 * CO
                w = f1 - f0
                WTf = WT[:, k0:k1].rearrange("p k c -> p (k c)")
                t = [tmix[:, n, 0:w] for n in range(4)]
                nc.vector.tensor_scalar_mul(t[0], bank_sb[:, 0, f0:f1],
                                            fm_sb[:, 0:1])
                nc.vector.tensor_scalar_mul(t[1], bank_sb[:, 1, f0:f1],
                                            fm_sb[:, 1:2])
                nc.scalar.mul(t[2], bank_sb[:, 2, f0:f1], fm_sb[:, 2:3])
                nc.scalar.mul(t[3], bank_sb[:, 3, f0:f1], fm_sb[:, 3:4])
                nc.vector.tensor_add(t[0], t[0], t[1])
                nc.vector.tensor_add(t[2], t[2], t[3])
                nc.vector.tensor_add(WTf, t[0], t[2])
                nc.vector.tensor_mul(sq[:, f0:f1], WTf[0:64], WTf[0:64])

            # ---- conv ----
            tiles_l = [(d, r0, nr) for d in D_ORDER for (r0, nr) in ROWSPLIT]
            quads = [(0, 0), (64, 0), (0, 64), (64, 64)]
            with tc.tile_pool(name="cpsum", bufs=8, space="PSUM") as cpsum:
                # demod scratch claims slot 0 so group slot reuse distance
                # stays >= 2 groups
                dps = cpsum.tile([128, 512], F32, tag="cps", name="dps")
                for ti in range(0, len(tiles_l), 4):
                    g = ti // 4
                    group = tiles_l[ti:ti + 4]
                    pss = [cpsum.tile([128, 512], F32, tag="cps",
                                      name=f"cps{j}")
                           for j in range(len(group))]
                    osbA = osb_pool.tile([128, 2, 11, W], F32, name="osbA")
                    osbs = [osbA[0:64, 0], osbA[0:64, 1],
                            osbA[64:128, 0], osbA[64:128, 1]]
                    taps_l = [_tile_taps(d, r0, nr) for (d, r0, nr) in group]
                    nwaves = max(len(t) for t in taps_l)
                    for i in range(nwaves):
                        if g == 0 and i == 14:
                            # demod sums over piece 0 (sq cols ready)
                            for k in range(KS):
                                nc.tensor.matmul(
                                    dps[0:1, 0:CO], lhsT=ones64,
                                    rhs=sq[:, k * CO:(k + 1) * CO],
                                    start=(k == 0), stop=False)
                        if g == 0 and i == 26:
                            for k in range(KS, 27):
                                nc.tensor.matmul(
                                    dps[0:1, 0:CO], lhsT=ones64,
                                    rhs=sq[:, k * CO:(k + 1) * CO],
                                    start=False, stop=(k == 26))
                            nc.scalar.activation(sstd, dps[0:1, 0:CO],
                                                 AF.Sqrt,
                                                 bias=eps_sb[:, 0:1])
                            nc.vector.reciprocal(demod_sb, sstd)
                        for j, (d, r0, nr) in enumerate(group):
                            taps = taps_l[j]
                            if i >= len(taps):
                                continue
                            kd, kh, kw, rlo, rcnt = taps[i]
                            rg, cp = quads[j]
                            koff = kd * 9 + kh * 3 + kw
                            n = rcnt * 34
                            c0 = (rlo - r0) * 34
                            off = (2 + (d + kd - 1) * PLANE
                                   + (rlo + kh) * 34 + kw - 1)
                            if rg:
                                off -= 1
                            nc.tensor.matmul(
                                pss[j][cp:cp + 64, c0:c0 + n],
                                lhsT=WT[rg:rg + 64, koff, :],
                                rhs=xbf[rg:rg + 64, off:off + n],
                                start=(i == 0), stop=(i == len(taps) - 1))
                    if g == 0:
                        # transpose demod to a per-partition column before
                        # this group's drains need it
                        nc.tensor.matmul(dps[0:CO, CO:CO + 1],
                                         lhsT=demod_sb, rhs=ones1[:, 0:1],
                                         start=True, stop=True)
                        nc.vector.tensor_copy(dmT, dps[0:CO, CO:CO + 1])
                    # drain: strided PSUM read -> compact SBUF -> one
                    # fully-contiguous store DMA per tile; engines and
                    # issue queues alternate to halve the serial cost
                    for j, (d, r0, nr) in enumerate(group):
                        cp = quads[j][1]
                        src = pss[j][cp:cp + 64, 0:nr * 34].rearrange(
                            "p (a b) -> p a b", b=34)[:, :, 1:W + 1]
                        if j >= 2:
                            nc.vector.tensor_scalar_mul(
                                osbs[j][:, 0:nr], src, dmT[:, 0:1])
                            nc.sync.dma_start(
                                out=out[:, d, r0:r0 + nr, :],
                                in_=osbs[j][:, 0:nr])
                        else:
                            nc.scalar.mul(osbs[j][:, 0:nr], src, dmT[:, 0:1])
                            nc.gpsimd.dma_start(
                                out=out[:, d, r0:r0 + nr, :],
                                in_=osbs[j][:, 0:nr])

    nc.compile()
    return nc


def _shard_inputs(x, w, filter_w, filter_b, mod_w, mod_b, bank):
    """Host-side input marshalling: per-core shards + replicated params in
    the layouts the kernel expects (padded bf16 x with both shifted
    copies; per-sample mix scalars; bf16 bank)."""
    import ml_dtypes
    wf = np.asarray(w, np.float32)
    logits = wf @ np.asarray(filter_w, np.float32).T + np.asarray(
        filter_b, np.float32)
    # softmax without the 1/sum: a uniform scale cancels through the
    # demodulation (exp of max-shifted logits for f32 safety)
    fwt = np.exp(logits - logits.max(axis=1, keepdims=True))  # [bs, bank]
    mod = wf @ np.asarray(mod_w, np.float32).T + np.asarray(
        mod_b, np.float32)  # [bs, ci]
    bank_h = np.ascontiguousarray(
        np.asarray(bank, np.float32).reshape(BANK, CO, CI, 27)
        .transpose(2, 0, 3, 1).reshape(CI, BANK, 27 * CO)
    ).astype(ml_dtypes.bfloat16)
    bank_h = np.ascontiguousarray(np.concatenate([bank_h, bank_h], axis=0))
    xf = np.asarray(x, np.float32)
    in_maps = []
    for i in range(NCORES):
        pad3 = np.zeros((CI, D, H + 2, W + 2), np.float32)
        pad3[:, :, 1:H + 1, 1:W + 1] = xf[i]
        fl = pad3.reshape(CI, -1).astype(ml_dtypes.bfloat16)
        xp = np.zeros((128, XCOLS), ml_dtypes.bfloat16)
        xp[0:64, 2:2 + D * PLANE] = fl
        xp[64:128, 1:1 + D * PLANE] = fl
        fm = mod[i][:, None] * fwt[i][None, :]  # [ci, bank]
        fm2 = np.ascontiguousarray(
            np.concatenate([fm, fm], axis=0), np.float32)
        in_maps.append({"xpad": xp, "fmh": fm2, "bankt": bank_h})
    return in_maps


def _run(inputs, trace=False):
    if "nc" not in _CACHE:
        _CACHE["nc"] = _build()
    nc = _CACHE["nc"]
    in_maps = _shard_inputs(**inputs)
    res = bass_utils.run_bass_kernel_spmd(
        nc, in_maps, core_ids=list(range(NCORES)), trace=trace)
    out = np.stack([res.results[i]["out"] for i in range(NCORES)])
    return out.astype(np.float32), res


def kernel(**inputs):
    out, _ = _run(inputs, trace=False)
    return out


# revision 23
# speedup vs baseline: 1.0402x; 1.0304x over previous
"""AdaptiveModulatedConv3d — 8-core TRN2 Bass kernel.

Problem (hardcoded): BS=8, C_IN=C_OUT=64, K=3, STYLE_DIM=512, BANK=4,
D=H=W=32, pad=1, stride=1, f32 in/out.

Sharding: pure data-parallel over batch — each of the 8 NeuronCores gets
one sample, builds its per-sample demodulated conv weights on-device, and
runs its own 3D conv. No collectives.

Per-core conv strategy: the 3x3x3 conv is decomposed into shifted matmuls
(contraction over C_IN=64) accumulating into PSUM. The PE 128x128 array is
quadrant-packed: row-groups 0/64 hold two copies of x (bf16, upper shifted
by +1 element), col-groups 0/64 compute two output tiles in separate PSUM
banks. Boundary taps (d and h) use narrowed-N matmuls instead of padding.

Latency layout: x ships from the host already padded + bf16 (both shifted
copies in one HBM buffer, chunk DMAs straight into SBUF). The per-sample
mix scalars fm[ci,n] = softmax_n(w@filter_w.T+fb)*mod[ci] are tiny
(BANK*CI values) and ride along as marshalled input; the bank arrives in
per-bank sub-DMAs so the DVE mix (bf16, two koff pieces) starts the moment
the first sub-bank lands. d-planes are visited in order [1,2,0,3,4,...] so
group 0 only needs the first WT piece. The demodulation (sum-of-squares
matmuls + rsqrt) is emitted interleaved into the early wave stream. Drains
read PSUM strided into compact SBUF tiles (fully contiguous store DMAs),
alternating ACT/DVE engines and GpSimd/SP issue queues.
"""

import numpy as np

import concourse.bass as bass
import concourse.tile as tile
from concourse import bacc, mybir
from concourse import bass_utils

F32 = mybir.dt.float32
BF16 = mybir.dt.bfloat16

BS = 8
CI = 64
CO = 64
SD = 512
BANK = 4
D = H = W = 32
EPS = 1e-8
NCORES = 8

PLANE = (H + 2) * (W + 2)  # 1156: h/w padded plane, flattened
XCOLS = 3 + D * PLANE
ROWSPLIT = [(0, 11), (11, 11), (22, 10)]  # h-row tiles per d-plane
KSPLIT = 18  # mix piece boundary (koff)
XCHUNKS = [(0, 2), (2, 4), (4, 8), (8, 12), (12, 16), (16, 20), (20, 24),
           (24, 28), (28, 32)]
D_ORDER = [1, 2, 0] + list(range(3, D))

_CACHE = {}


def _tile_taps(d, r0, nr):
    """Valid taps for tile (d, r0, nr) with h-boundary narrowing: rows
    whose x_pad source row is padding are excluded (their contribution is
    zero), so no row-border zeroing is ever needed."""
    taps = []
    for kd in range(3):
        if not (0 <= d + kd - 1 <= D - 1):
            continue
        for kh in range(3):
            rlo = max(r0, 1 - kh)
            rhi = min(r0 + nr - 1, 32 - kh)
            for kw in range(3):
                taps.append((kd, kh, kw, rlo, rhi - rlo + 1))
    return taps


def _build():
    nc = bacc.Bacc("TRN2", target_bir_lowering=False, debug=False)
    xpad = nc.dram_tensor("xpad", [128, XCOLS], BF16,
                          kind="ExternalInput").ap()
    fmh = nc.dram_tensor("fmh", [128, BANK], F32, kind="ExternalInput").ap()
    bankt = nc.dram_tensor("bankt", [128, BANK, 27 * CO], BF16,
                           kind="ExternalInput").ap()
    out = nc.dram_tensor("out", [CO, D, H, W], F32, kind="ExternalOutput").ap()

    AF = mybir.ActivationFunctionType
    MULT, ADD = mybir.AluOpType.mult, mybir.AluOpType.add
    KS = KSPLIT

    with tile.TileContext(nc) as tc:
        with tc.tile_pool(name="singles", bufs=1) as singles, \
             tc.tile_pool(name="osb", bufs=10) as osb_pool:

            # fm and bank arrive duplicated on both partition halves, so
            # the mix runs on all 128 DVE lanes and writes both WT copies
            # at once (no upper-half dup DMA). DMA issue order on the sync
            # queue puts the mix's critical chain ahead of the bulk of x.
            fm_sb = singles.tile([128, BANK], F32)
            nc.sync.dma_start(out=fm_sb, in_=fmh)
            bank_sb = singles.tile([128, BANK, 27 * CO], BF16)
            xbf = singles.tile([128, XCOLS], BF16)

            def emit_xchunk(ci):
                p0, p1 = XCHUNKS[ci]
                a = 0 if p0 == 0 else 1 + p0 * PLANE
                b = XCOLS if p1 == D else 2 + p1 * PLANE
                nc.sync.dma_start(out=xbf[:, a:b], in_=xpad[:, a:b])

            # bank piece-0 sub-DMAs split across the sync and ACT issue
            # queues (issue cost ~0.7us each serializes per queue); x
            # chunks 0-1 and bank piece 1 ride ACT, the rest sync
            def bank_n(eng, n):
                eng.dma_start(out=bank_sb[:, n, 0:KS * CO],
                              in_=bankt[:, n, 0:KS * CO])

            bank_n(nc.sync, 0)
            bank_n(nc.scalar, 2)
            bank_n(nc.sync, 1)
            bank_n(nc.scalar, 3)
            nc.scalar.dma_start(out=xbf[:, 0:2 + 2 * PLANE],
                                in_=xpad[:, 0:2 + 2 * PLANE])
            nc.scalar.dma_start(
                out=xbf[:, 1 + 2 * PLANE:2 + 4 * PLANE],
                in_=xpad[:, 1 + 2 * PLANE:2 + 4 * PLANE])
            nc.scalar.dma_start(out=bank_sb[:, :, KS * CO:27 * CO],
                                in_=bankt[:, :, KS * CO:27 * CO])
            emit_xchunk(2)
            tmix = singles.tile([128, 4, KS * CO], BF16)
            # x chunks 3+ are issued from inside the conv group loop so
            # their DMA traffic self-paces behind the store DMAs instead
            # of racing the bank/WT critical path at startup
            xchunk_at = {2: [3], 5: [4], 8: [5], 11: [6], 14: [7], 17: [8]}

            warm = singles.tile([1, 1], F32)
            nc.vector.memset(warm, 0.0)
            ones1 = singles.tile([1, 64], F32)
            nc.vector.memset(ones1, 1.0)
            ones64 = singles.tile([64, 1], BF16)
            nc.vector.memset(ones64, 1.0)
            eps_sb = singles.tile([1, 1], F32)
            nc.vector.memset(eps_sb, EPS)
            wdum = singles.tile([1, 512], BF16)
            nc.vector.memset(wdum, 1.0)
            nc.scalar.activation(warm, warm, AF.Sqrt)  # table warm

            # PE warm-up: ~3.5us of dummy matmuls releases the HAM clock
            # throttle (1.2 -> 2.4 GHz) just before the conv stream starts
            with tc.tile_pool(name="hpsum", bufs=1, space="PSUM") as hpsum:
                hps = hpsum.tile([1, 512], F32, tag="hps")
                for _ in range(8):
                    nc.tensor.matmul(hps, lhsT=ones64[0:1, 0:1], rhs=wdum,
                                     start=True, stop=True)

            WT = singles.tile([128, 27, CO], BF16)
            sq = singles.tile([CI, 27 * CO], BF16)
            sstd = singles.tile([1, CO], F32)
            demod_sb = singles.tile([1, CO], F32)
            dmT = singles.tile([CO, 1], F32)

            # bank mix on all 128 DVE lanes: WT[ci(+64),koff,co] =
            # sum_n fm[ci,n]*bank[ci,n,...], bf16. Mul-tree formulation:
            # tensor_scalar muls run at the DVE 4x / ACT packed rate and
            # bf16 tensor_tensor adds at 2x, vs 1x for the naive
            # scalar_tensor_tensor chain; the two ACT muls overlap DVE.
            for (k0, k1) in ((0, KS), (KS, 27)):
                f0, f1 = k0 * CO, k1 * CO
                w = f1 - f0
                WTf = WT[:, k0:k1].rearrange("p k c -> p (k c)")
                t = [tmix[:, n, 0:w] for n in range(4)]
                for n in range(4):
                    nc.vector.tensor_scalar_mul(t[n], bank_sb[:, n, f0:f1],
                                                fm_sb[:, n:n + 1])
                nc.vector.tensor_add(t[0], t[0], t[1])
                nc.vector.tensor_add(t[2], t[2], t[3])
                nc.vector.tensor_add(WTf, t[0], t[2])
                nc.vector.tensor_mul(sq[:, f0:f1], WTf[0:64], WTf[0:64])

            # ---- conv ----
            tiles_l = [(d, r0, nr) for d in D_ORDER for (r0, nr) in ROWSPLIT]
            quads = [(0, 0), (64, 0), (0, 64), (64, 64)]
            with tc.tile_pool(name="cpsum", bufs=8, space="PSUM") as cpsum:
                # demod scratch claims slot 0 so group slot reuse distance
                # stays >= 2 groups
                dps = cpsum.tile([128, 512], F32, tag="cps", name="dps")
                for ti in range(0, len(tiles_l), 4):
                    g = ti // 4
                    group = tiles_l[ti:ti + 4]
                    pss = [cpsum.tile([128, 512], F32, tag="cps",
                                      name=f"cps{j}")
                           for j in range(len(group))]
                    osbA = osb_pool.tile([128, 2, 11, W], F32, name="osbA")
                    osbs = [osbA[0:64, 0], osbA[0:64, 1],
                            osbA[64:128, 0], osbA[64:128, 1]]
                    taps_l = [_tile_taps(d, r0, nr) for (d, r0, nr) in group]
                    nwaves = max(len(t) for t in taps_l)
                    for i in range(nwaves):
                        if g == 0 and i == 14:
                            # demod sums over piece 0 (sq cols ready)
                            for k in range(KS):
                                nc.tensor.matmul(
                                    dps[0:1, 0:CO], lhsT=ones64,
                                    rhs=sq[:, k * CO:(k + 1) * CO],
                                    start=(k == 0), stop=False)
                        if g == 0 and i == 26:
                            for k in range(KS, 27):
                                nc.tensor.matmul(
                                    dps[0:1, 0:CO], lhsT=ones64,
                                    rhs=sq[:, k * CO:(k + 1) * CO],
                                    start=False, stop=(k == 26))
                            nc.scalar.activation(sstd, dps[0:1, 0:CO],
                                                 AF.Sqrt,
                                                 bias=eps_sb[:, 0:1])
                            nc.vector.reciprocal(demod_sb, sstd)
                        for j, (d, r0, nr) in enumerate(group):
                            taps = taps_l[j]
                            if i >= len(taps):
                                continue
                            kd, kh, kw, rlo, rcnt = taps[i]
                            rg, cp = quads[j]
                            koff = kd * 9 + kh * 3 + kw
                            n = rcnt * 34
                            c0 = (rlo - r0) * 34
                            off = (2 + (d + kd - 1) * PLANE
                                   + (rlo + kh) * 34 + kw - 1)
                            if rg:
                                off -= 1
                            nc.tensor.matmul(
                                pss[j][cp:cp + 64, c0:c0 + n],
                                lhsT=WT[rg:rg + 64, koff, :],
                                rhs=xbf[rg:rg + 64, off:off + n],
                                start=(i == 0), stop=(i == len(taps) - 1))
                    if g == 0:
                        # transpose demod to a per-partition column before
                        # this group's drains need it
                        nc.tensor.matmul(dps[0:CO, CO:CO + 1],
                                         lhsT=demod_sb, rhs=ones1[:, 0:1],
                                         start=True, stop=True)
                        nc.vector.tensor_copy(dmT, dps[0:CO, CO:CO + 1])
                    # drain: strided PSUM read -> compact SBUF -> one
                    # fully-contiguous store DMA per tile; engines and
                    # issue queues alternate to halve the serial cost
                    for j, (d, r0, nr) in enumerate(group):
                        cp = quads[j][1]
                        src = pss[j][cp:cp + 64, 0:nr * 34].rearrange(
                            "p (a b) -> p a b", b=34)[:, :, 1:W + 1]
                        if j >= 2:
                            nc.vector.tensor_scalar_mul(
                                osbs[j][:, 0:nr], src, dmT[:, 0:1])
                            nc.sync.dma_start(
                                out=out[:, d, r0:r0 + nr, :],
                                in_=osbs[j][:, 0:nr])
                        else:
                            nc.scalar.mul(osbs[j][:, 0:nr], src, dmT[:, 0:1])
                            nc.gpsimd.dma_start(
                                out=out[:, d, r0:r0 + nr, :],
                                in_=osbs[j][:, 0:nr])
                    for ci in xchunk_at.get(g, ()):
                        emit_xchunk(ci)

    nc.compile()
    return nc


def _shard_inputs(x, w, filter_w, filter_b, mod_w, mod_b, bank):
    """Host-side input marshalling: per-core shards + replicated params in
    the layouts the kernel expects (padded bf16 x with both shifted
    copies; per-sample mix scalars; bf16 bank)."""
    import ml_dtypes
    wf = np.asarray(w, np.float32)
    logits = wf @ np.asarray(filter_w, np.float32).T + np.asarray(
        filter_b, np.float32)
    # softmax without the 1/sum: a uniform scale cancels through the
    # demodulation (exp of max-shifted logits for f32 safety)
    fwt = np.exp(logits - logits.max(axis=1, keepdims=True))  # [bs, bank]
    mod = wf @ np.asarray(mod_w, np.float32).T + np.asarray(
        mod_b, np.float32)  # [bs, ci]
    bank_h = np.ascontiguousarray(
        np.asarray(bank, np.float32).reshape(BANK, CO, CI, 27)
        .transpose(2, 0, 3, 1).reshape(CI, BANK, 27 * CO)
    ).astype(ml_dtypes.bfloat16)
    bank_h = np.ascontiguousarray(np.concatenate([bank_h, bank_h], axis=0))
    xf = np.asarray(x, np.float32)
    in_maps = []
    for i in range(NCORES):
        pad3 = np.zeros((CI, D, H + 2, W + 2), np.float32)
        pad3[:, :, 1:H + 1, 1:W + 1] = xf[i]
        fl = pad3.reshape(CI, -1).astype(ml_dtypes.bfloat16)
        xp = np.zeros((128, XCOLS), ml_dtypes.bfloat16)
        xp[0:64, 2:2 + D * PLANE] = fl
        xp[64:128, 1:1 + D * PLANE] = fl
        fm = mod[i][:, None] * fwt[i][None, :]  # [ci, bank]
        fm2 = np.ascontiguousarray(
            np.concatenate([fm, fm], axis=0), np.float32)
        in_maps.append({"xpad": xp, "fmh": fm2, "bankt": bank_h})
    return in_maps


def _run(inputs, trace=False):
    if "nc" not in _CACHE:
        _CACHE["nc"] = _build()
    nc = _CACHE["nc"]
    in_maps = _shard_inputs(**inputs)
    res = bass_utils.run_bass_kernel_spmd(
        nc, in_maps, core_ids=list(range(NCORES)), trace=trace)
    out = np.stack([res.results[i]["out"] for i in range(NCORES)])
    return out.astype(np.float32), res


def kernel(**inputs):
    out, _ = _run(inputs, trace=False)
    return out


# revision 26
# speedup vs baseline: 1.0458x; 1.0053x over previous
"""AdaptiveModulatedConv3d — 8-core TRN2 Bass kernel.

Problem (hardcoded): BS=8, C_IN=C_OUT=64, K=3, STYLE_DIM=512, BANK=4,
D=H=W=32, pad=1, stride=1, f32 in/out.

Sharding: pure data-parallel over batch — each of the 8 NeuronCores gets
one sample, builds its per-sample demodulated conv weights on-device, and
runs its own 3D conv. No collectives.

Per-core conv strategy: the 3x3x3 conv is decomposed into shifted matmuls
(contraction over C_IN=64) accumulating into PSUM. The PE 128x128 array is
quadrant-packed: row-groups 0/64 hold two copies of x (bf16, upper shifted
by +1 element), col-groups 0/64 compute two output tiles in separate PSUM
banks. Boundary taps (d and h) use narrowed-N matmuls instead of padding.

Latency layout: x ships from the host already padded + bf16 (both shifted
copies in one HBM buffer, chunk DMAs straight into SBUF). The per-sample
mix scalars fm[ci,n] = softmax_n(w@filter_w.T+fb)*mod[ci] are tiny
(BANK*CI values) and ride along as marshalled input; the bank arrives in
per-bank sub-DMAs so the DVE mix (bf16, two koff pieces) starts the moment
the first sub-bank lands. d-planes are visited in order [1,2,0,3,4,...] so
group 0 only needs the first WT piece. The demodulation (sum-of-squares
matmuls + rsqrt) is emitted interleaved into the early wave stream. Drains
read PSUM strided into compact SBUF tiles (fully contiguous store DMAs),
alternating ACT/DVE engines and GpSimd/SP issue queues.
"""

import numpy as np

import concourse.bass as bass
import concourse.tile as tile
from concourse import bacc, mybir
from concourse import bass_utils

F32 = mybir.dt.float32
BF16 = mybir.dt.bfloat16

BS = 8
CI = 64
CO = 64
SD = 512
BANK = 4
D = H = W = 32
EPS = 1e-8
NCORES = 8

PLANE = (H + 2) * (W + 2)  # 1156: h/w padded plane, flattened
XCOLS = 3 + D * PLANE
ROWSPLIT = [(0, 11), (11, 11), (22, 10)]  # h-row tiles per d-plane
KSPLIT = 18  # mix piece boundary (koff)
XCHUNKS = [(0, 2), (2, 4), (4, 8), (8, 12), (12, 16), (16, 20), (20, 24),
           (24, 28), (28, 32)]
D_ORDER = [1, 2, 0] + list(range(3, D))

_CACHE = {}


def _tile_taps(d, r0, nr):
    """Valid taps for tile (d, r0, nr) with h-boundary narrowing: rows
    whose x_pad source row is padding are excluded (their contribution is
    zero), so no row-border zeroing is ever needed."""
    taps = []
    for kd in range(3):
        if not (0 <= d + kd - 1 <= D - 1):
            continue
        for kh in range(3):
            rlo = max(r0, 1 - kh)
            rhi = min(r0 + nr - 1, 32 - kh)
            for kw in range(3):
                taps.append((kd, kh, kw, rlo, rhi - rlo + 1))
    return taps


def _build():
    nc = bacc.Bacc("TRN2", target_bir_lowering=False, debug=False)
    xpad = nc.dram_tensor("xpad", [128, XCOLS], BF16,
                          kind="ExternalInput").ap()
    fmh = nc.dram_tensor("fmh", [128, BANK], F32, kind="ExternalInput").ap()
    bankt = nc.dram_tensor("bankt", [128, BANK, 27 * CO], BF16,
                           kind="ExternalInput").ap()
    out = nc.dram_tensor("out", [CO, D, H, W], F32, kind="ExternalOutput").ap()

    AF = mybir.ActivationFunctionType
    MULT, ADD = mybir.AluOpType.mult, mybir.AluOpType.add
    KS = KSPLIT

    with tile.TileContext(nc) as tc:
        with tc.tile_pool(name="singles", bufs=1) as singles, \
             tc.tile_pool(name="osb", bufs=10) as osb_pool:

            # fm and bank arrive duplicated on both partition halves, so
            # the mix runs on all 128 DVE lanes and writes both WT copies
            # at once (no upper-half dup DMA). DMA issue order on the sync
            # queue puts the mix's critical chain ahead of the bulk of x.
            fm_sb = singles.tile([128, BANK], F32)
            nc.sync.dma_start(out=fm_sb, in_=fmh)
            bank_sb = singles.tile([128, BANK, 27 * CO], BF16)
            xbf = singles.tile([128, XCOLS], BF16)

            def emit_xchunk(ci):
                p0, p1 = XCHUNKS[ci]
                a = 0 if p0 == 0 else 1 + p0 * PLANE
                b = XCOLS if p1 == D else 2 + p1 * PLANE
                nc.sync.dma_start(out=xbf[:, a:b], in_=xpad[:, a:b])

            # bank piece-0 sub-DMAs split across the sync and ACT issue
            # queues (issue cost ~0.7us each serializes per queue); x
            # chunks 0-1 and bank piece 1 ride ACT, the rest sync
            def bank_n(eng, n):
                eng.dma_start(out=bank_sb[:, n, 0:KS * CO],
                              in_=bankt[:, n, 0:KS * CO])

            bank_n(nc.sync, 0)
            bank_n(nc.scalar, 2)
            bank_n(nc.sync, 1)
            bank_n(nc.scalar, 3)
            nc.sync.dma_start(out=bank_sb[:, :, KS * CO:27 * CO],
                              in_=bankt[:, :, KS * CO:27 * CO])
            nc.scalar.dma_start(out=xbf[:, 0:2 + 2 * PLANE],
                                in_=xpad[:, 0:2 + 2 * PLANE])
            nc.scalar.dma_start(
                out=xbf[:, 1 + 2 * PLANE:2 + 4 * PLANE],
                in_=xpad[:, 1 + 2 * PLANE:2 + 4 * PLANE])
            tmix = singles.tile([128, 4, KS * CO], BF16)
            # x chunks 2+ are issued from inside the conv group loop so
            # their DMA traffic self-paces behind the store DMAs instead
            # of racing the bank/WT critical path at startup
            xchunk_at = {0: [2], 2: [3], 5: [4], 8: [5], 11: [6], 14: [7],
                         17: [8]}

            warm = singles.tile([1, 1], F32)
            nc.vector.memset(warm, 0.0)
            ones1 = singles.tile([1, 64], F32)
            nc.vector.memset(ones1, 1.0)
            ones64 = singles.tile([64, 1], BF16)
            nc.vector.memset(ones64, 1.0)
            eps_sb = singles.tile([1, 1], F32)
            nc.vector.memset(eps_sb, EPS)
            wdum = singles.tile([1, 512], BF16)
            nc.vector.memset(wdum, 1.0)
            nc.scalar.activation(warm, warm, AF.Sqrt)  # table warm

            # PE warm-up: ~3.5us of dummy matmuls releases the HAM clock
            # throttle (1.2 -> 2.4 GHz) just before the conv stream starts
            with tc.tile_pool(name="hpsum", bufs=1, space="PSUM") as hpsum:
                hps = hpsum.tile([1, 512], F32, tag="hps")
                for _ in range(8):
                    nc.tensor.matmul(hps, lhsT=ones64[0:1, 0:1], rhs=wdum,
                                     start=True, stop=True)

            WT = singles.tile([128, 27, CO], BF16)
            sq = singles.tile([CI, 27 * CO], BF16)
            sstd = singles.tile([1, CO], F32)
            demod_sb = singles.tile([1, CO], F32)
            dmT = singles.tile([CO, 1], F32)

            # bank mix on all 128 DVE lanes: WT[ci(+64),koff,co] =
            # sum_n fm[ci,n]*bank[ci,n,...], bf16. Mul-tree formulation:
            # tensor_scalar muls run at the DVE 4x / ACT packed rate and
            # bf16 tensor_tensor adds at 2x, vs 1x for the naive
            # scalar_tensor_tensor chain; the two ACT muls overlap DVE.
            for (k0, k1) in ((0, KS), (KS, 27)):
                f0, f1 = k0 * CO, k1 * CO
                w = f1 - f0
                WTf = WT[:, k0:k1].rearrange("p k c -> p (k c)")
                t = [tmix[:, n, 0:w] for n in range(4)]
                for n in range(4):
                    nc.vector.tensor_scalar_mul(t[n], bank_sb[:, n, f0:f1],
                                                fm_sb[:, n:n + 1])
                nc.vector.tensor_add(t[0], t[0], t[1])
                nc.vector.tensor_add(t[2], t[2], t[3])
                nc.vector.tensor_add(WTf, t[0], t[2])
                # squares for the demod sums off the critical DVE path
                nc.gpsimd.tensor_mul(sq[:, f0:f1], WTf[0:64], WTf[0:64])

            # ---- conv ----
            tiles_l = [(d, r0, nr) for d in D_ORDER for (r0, nr) in ROWSPLIT]
            quads = [(0, 0), (64, 0), (0, 64), (64, 64)]
            with tc.tile_pool(name="cpsum", bufs=8, space="PSUM") as cpsum:
                # demod scratch claims slot 0 so group slot reuse distance
                # stays >= 2 groups
                dps = cpsum.tile([128, 512], F32, tag="cps", name="dps")
                for ti in range(0, len(tiles_l), 4):
                    g = ti // 4
                    group = tiles_l[ti:ti + 4]
                    pss = [cpsum.tile([128, 512], F32, tag="cps",
                                      name=f"cps{j}")
                           for j in range(len(group))]
                    osbA = osb_pool.tile([128, 2, 11, W], F32, name="osbA")
                    osbs = [osbA[0:64, 0], osbA[0:64, 1],
                            osbA[64:128, 0], osbA[64:128, 1]]
                    taps_l = [_tile_taps(d, r0, nr) for (d, r0, nr) in group]
                    nwaves = max(len(t) for t in taps_l)
                    for i in range(nwaves):
                        if g == 0 and i == 19:
                            # demod sums over piece 0 (sq cols ready)
                            for k in range(KS):
                                nc.tensor.matmul(
                                    dps[0:1, 0:CO], lhsT=ones64,
                                    rhs=sq[:, k * CO:(k + 1) * CO],
                                    start=(k == 0), stop=False)
                        if g == 0 and i == 26:
                            for k in range(KS, 27):
                                nc.tensor.matmul(
                                    dps[0:1, 0:CO], lhsT=ones64,
                                    rhs=sq[:, k * CO:(k + 1) * CO],
                                    start=False, stop=(k == 26))
                            nc.scalar.activation(sstd, dps[0:1, 0:CO],
                                                 AF.Sqrt,
                                                 bias=eps_sb[:, 0:1])
                            nc.vector.reciprocal(demod_sb, sstd)
                        for j, (d, r0, nr) in enumerate(group):
                            taps = taps_l[j]
                            if i >= len(taps):
                                continue
                            kd, kh, kw, rlo, rcnt = taps[i]
                            rg, cp = quads[j]
                            koff = kd * 9 + kh * 3 + kw
                            n = rcnt * 34
                            c0 = (rlo - r0) * 34
                            off = (2 + (d + kd - 1) * PLANE
                                   + (rlo + kh) * 34 + kw - 1)
                            if rg:
                                off -= 1
                            nc.tensor.matmul(
                                pss[j][cp:cp + 64, c0:c0 + n],
                                lhsT=WT[rg:rg + 64, koff, :],
                                rhs=xbf[rg:rg + 64, off:off + n],
                                start=(i == 0), stop=(i == len(taps) - 1))
                    if g == 0:
                        # transpose demod to a per-partition column before
                        # this group's drains need it
                        nc.tensor.matmul(dps[0:CO, CO:CO + 1],
                                         lhsT=demod_sb, rhs=ones1[:, 0:1],
                                         start=True, stop=True)
                        nc.vector.tensor_copy(dmT, dps[0:CO, CO:CO + 1])
                    # drain: strided PSUM read -> compact SBUF -> one
                    # fully-contiguous store DMA per tile; engines and
                    # issue queues alternate to halve the serial cost
                    for j, (d, r0, nr) in enumerate(group):
                        cp = quads[j][1]
                        src = pss[j][cp:cp + 64, 0:nr * 34].rearrange(
                            "p (a b) -> p a b", b=34)[:, :, 1:W + 1]
                        if j >= 2:
                            nc.vector.tensor_scalar_mul(
                                osbs[j][:, 0:nr], src, dmT[:, 0:1])
                            nc.sync.dma_start(
                                out=out[:, d, r0:r0 + nr, :],
                                in_=osbs[j][:, 0:nr])
                        else:
                            nc.scalar.mul(osbs[j][:, 0:nr], src, dmT[:, 0:1])
                            nc.gpsimd.dma_start(
                                out=out[:, d, r0:r0 + nr, :],
                                in_=osbs[j][:, 0:nr])
                    for ci in xchunk_at.get(g, ()):
                        emit_xchunk(ci)

    nc.compile()
    return nc


def _shard_inputs(x, w, filter_w, filter_b, mod_w, mod_b, bank):
    """Host-side input marshalling: per-core shards + replicated params in
    the layouts the kernel expects (padded bf16 x with both shifted
    copies; per-sample mix scalars; bf16 bank)."""
    import ml_dtypes
    wf = np.asarray(w, np.float32)
    logits = wf @ np.asarray(filter_w, np.float32).T + np.asarray(
        filter_b, np.float32)
    # softmax without the 1/sum: a uniform scale cancels through the
    # demodulation (exp of max-shifted logits for f32 safety)
    fwt = np.exp(logits - logits.max(axis=1, keepdims=True))  # [bs, bank]
    mod = wf @ np.asarray(mod_w, np.float32).T + np.asarray(
        mod_b, np.float32)  # [bs, ci]
    bank_h = np.ascontiguousarray(
        np.asarray(bank, np.float32).reshape(BANK, CO, CI, 27)
        .transpose(2, 0, 3, 1).reshape(CI, BANK, 27 * CO)
    ).astype(ml_dtypes.bfloat16)
    bank_h = np.ascontiguousarray(np.concatenate([bank_h, bank_h], axis=0))
    xf = np.asarray(x, np.float32)
    in_maps = []
    for i in range(NCORES):
        pad3 = np.zeros((CI, D, H + 2, W + 2), np.float32)
        pad3[:, :, 1:H + 1, 1:W + 1] = xf[i]
        fl = pad3.reshape(CI, -1).astype(ml_dtypes.bfloat16)
        xp = np.zeros((128, XCOLS), ml_dtypes.bfloat16)
        xp[0:64, 2:2 + D * PLANE] = fl
        xp[64:128, 1:1 + D * PLANE] = fl
        fm = mod[i][:, None] * fwt[i][None, :]  # [ci, bank]
        fm2 = np.ascontiguousarray(
            np.concatenate([fm, fm], axis=0), np.float32)
        in_maps.append({"xpad": xp, "fmh": fm2, "bankt": bank_h})
    return in_maps


def _run(inputs, trace=False):
    if "nc" not in _CACHE:
        _CACHE["nc"] = _build()
    nc = _CACHE["nc"]
    in_maps = _shard_inputs(**inputs)
    res = bass_utils.run_bass_kernel_spmd(
        nc, in_maps, core_ids=list(range(NCORES)), trace=trace)
    out = np.stack([res.results[i]["out"] for i in range(NCORES)])
    return out.astype(np.float32), res


def kernel(**inputs):
    out, _ = _run(inputs, trace=False)
    return out
